# revision 1
# baseline (speedup 1.0000x reference)
"""Trainium2 Bass kernel for nn_AffineTransformer_6442450944616.

kernel(**inputs) takes the FULL unsharded inputs (affine_outs [2048,6],
fill_alpha [2048,64,64], stroke_alpha [2048,64,64], targetsize [1,2048,2])
and returns (fill_out, stroke_out), each [2048,128,128] float32 — matching
reference.reference().

Strategy: pure data parallel over the sample axis; 256 samples per
NeuronCore on 8 cores.  On-device math per sample i, output pixel j
(p=j//128, q=j%128):
    ix(j) = t00*q + t01*p + Cx ;  iy(j) = t10*q + t11*p + Cy
    out[j] = sum_{x,y in payload} relu(1-|ix-x|)*relu(1-|iy-y|)*img[y,x]
which is exactly bilinear sampling with zeros padding (out-of-range taps
have zero hat weight or zero image value).  The contraction and the
hat-argument construction run on the PE (tensor engine) from small
host-precomputed per-sample stationary matrices; |.| and relu run on
ACT; the hat_y multiply runs on DVE.
"""
import numpy as np

import concourse.bass as bass
import concourse.bacc as bacc
import concourse.tile as tile
import concourse.mybir as mybir
from concourse.bass_utils import run_bass_kernel_spmd

F32 = mybir.dt.float32
AL = mybir.AluOpType
ACTF = mybir.ActivationFunctionType

N = 2048
NCORES = 8
NS = N // NCORES      # 256 samples per core
P = 128
NPIX = P * P
CH = 2048
NCH = NPIX // CH


def _build(ns: int):
    nc = bacc.Bacc("TRN2", target_bir_lowering=False, debug=False)
    ibt_d = nc.dram_tensor("ibt", [ns, 64, P], F32, kind="ExternalInput")
    wx_d = nc.dram_tensor("wx", [ns, 3, 64], F32, kind="ExternalInput")
    wy_d = nc.dram_tensor("wy", [ns, 3, P], F32, kind="ExternalInput")
    j_d = nc.dram_tensor("jconst", [3, NPIX], F32, kind="ExternalInput")
    ones_d = nc.dram_tensor("ones2", [P, 2], F32, kind="ExternalInput")
    fo_d = nc.dram_tensor("fill_out", [ns, P, P], F32, kind="ExternalOutput")
    so_d = nc.dram_tensor("stroke_out", [ns, P, P], F32, kind="ExternalOutput")

    with tile.TileContext(nc) as tc:
        with tc.tile_pool(name="const", bufs=1) as cpool, \
             tc.tile_pool(name="work", bufs=2) as pool, \
             tc.tile_pool(name="ps", bufs=1, space="PSUM") as psum:
            jt = cpool.tile([3, NPIX], F32, tag="jt")
            ones2 = cpool.tile([P, 2], F32, tag="ones2")
            nc.sync.dma_start(out=jt[:], in_=j_d[:, :])
            nc.sync.dma_start(out=ones2[:], in_=ones_d[:, :])

            def body(i):
                wxt = pool.tile([3, 64], F32, tag="wxt", name=f"wxt{i}")
                wyt = pool.tile([3, P], F32, tag="wyt", name=f"wyt{i}")
                ibt = pool.tile([64, P], F32, tag="ibt", name=f"ibt{i}")
                nc.sync.dma_start(out=wxt[:], in_=wx_d[bass.ds(i, 1), :, :])
                nc.sync.dma_start(out=wyt[:], in_=wy_d[bass.ds(i, 1), :, :])
                nc.sync.dma_start(out=ibt[:], in_=ibt_d[bass.ds(i, 1), :, :])
                for c in range(NCH):
                    dx = psum.tile([64, CH], F32, tag="D", name=f"dx{i}_{c}")
                    for h in range(4):
                        hs = slice(h * 512, (h + 1) * 512)
                        nc.tensor.matmul(out=dx[:, hs], lhsT=wxt[:],
                                         rhs=jt[:, c * CH + h * 512: c * CH + (h + 1) * 512],
                                         start=True, stop=True)
                    ax = pool.tile([64, CH], F32, tag="ax", name=f"ax{i}_{c}")
                    nc.scalar.activation(out=ax[:], in_=dx[:], func=ACTF.Abs)
                    hx = pool.tile([64, CH], F32, tag="hx", name=f"hx{i}_{c}")
                    nc.vector.tensor_scalar(hx[:], ax[:], -1.0, 1.0, AL.mult, AL.add)
                    nc.vector.tensor_scalar(hx[:], hx[:], 0.0, None, AL.max)
                    cc = psum.tile([P, CH], F32, tag="C", name=f"cc{i}_{c}")
                    for h in range(4):
                        hs = slice(h * 512, (h + 1) * 512)
                        nc.tensor.matmul(out=cc[:, hs], lhsT=ibt[:], rhs=hx[:, hs],
                                         start=True, stop=True)
                    dy = psum.tile([P, CH], F32, tag="D", name=f"dy{i}_{c}")
                    for h in range(4):
                        hs = slice(h * 512, (h + 1) * 512)
                        nc.tensor.matmul(out=dy[:, hs], lhsT=wyt[:],
                                         rhs=jt[:, c * CH + h * 512: c * CH + (h + 1) * 512],
                                         start=True, stop=True)
                    ay = pool.tile([P, CH], F32, tag="ay", name=f"ay{i}_{c}")
                    nc.scalar.activation(out=ay[:], in_=dy[:], func=ACTF.Abs)
                    hy = pool.tile([P, CH], F32, tag="hy", name=f"hy{i}_{c}")
                    nc.scalar.activation(out=hy[:], in_=ay[:], func=ACTF.Relu,
                                         scale=-1.0, bias=1.0)
                    mm = pool.tile([P, CH], F32, tag="mm", name=f"mm{i}_{c}")
                    nc.vector.tensor_tensor(mm[:], cc[:], hy[:], AL.mult)
                    oo = psum.tile([2, CH], F32, tag="D", name=f"oo{i}_{c}")
                    for h in range(4):
                        hs = slice(h * 512, (h + 1) * 512)
                        nc.tensor.matmul(out=oo[:, hs], lhsT=ones2[:], rhs=mm[:, hs],
                                         start=True, stop=True)
                    o2 = pool.tile([2, CH], F32, tag="o2", name=f"o2{i}_{c}")
                    nc.scalar.copy(out=o2[:], in_=oo[:])
                    for r, dd in ((0, fo_d), (1, so_d)):
                        nc.sync.dma_start(
                            out=dd[bass.ds(i, 1), 16 * c:16 * (c + 1), :],
                            in_=o2[r:r + 1, :])

            with tc.For_i(0, ns, 1) as i:
                body(i)
    nc.compile()
    return nc


def _host_prep(affine_outs, fill, stroke):
    n = affine_outs.shape[0]
    a = affine_outs.astype(np.float64)
    sig = lambda v: 1.0 / (1.0 + np.exp(-v))
    t00 = 2 * sig(a[:, 0]); t11 = 2 * sig(a[:, 1])
    t01 = 2 * np.tanh(a[:, 2]); t10 = 2 * np.tanh(a[:, 3])
    t02 = np.tanh(a[:, 4]); t12 = np.tanh(a[:, 5])
    cx = (t00 + t01) * (0.5 - 64.0) + 64.0 * t02 + 63.5
    cy = (t10 + t11) * (0.5 - 64.0) + 64.0 * t12 + 63.5
    wx = np.zeros((n, 3, 64), np.float32)
    wy = np.zeros((n, 3, P), np.float32)
    m64 = np.arange(64)
    wx[:, 0, :] = t01[:, None]
    wx[:, 1, :] = t00[:, None]
    wx[:, 2, :] = cx[:, None] - (32.0 + m64)[None, :]
    my = np.concatenate([m64, m64])
    wy[:, 0, :] = t11[:, None]
    wy[:, 1, :] = t10[:, None]
    wy[:, 2, :] = cy[:, None] - (32.0 + my)[None, :]
    ibt = np.ascontiguousarray(np.concatenate(
        [np.asarray(fill).transpose(0, 2, 1),
         np.asarray(stroke).transpose(0, 2, 1)], axis=2).astype(np.float32))
    pj = (np.arange(NPIX) // P).astype(np.float32)
    qj = (np.arange(NPIX) % P).astype(np.float32)
    jconst = np.stack([pj, qj, np.ones(NPIX, np.float32)])
    ones2 = np.zeros((P, 2), np.float32)
    ones2[:64, 0] = 1.0
    ones2[64:, 1] = 1.0
    return ibt, wx, wy, jconst, ones2


_NC_CACHE = {}


def _get_nc():
    if "nc" not in _NC_CACHE:
        _NC_CACHE["nc"] = _build(NS)
    return _NC_CACHE["nc"]


def kernel(affine_outs, fill_alpha, stroke_alpha, targetsize):
    affine_outs = np.asarray(affine_outs, dtype=np.float32)
    fill_alpha = np.asarray(fill_alpha, dtype=np.float32)
    stroke_alpha = np.asarray(stroke_alpha, dtype=np.float32)
    ibt, wx, wy, jconst, ones2 = _host_prep(affine_outs, fill_alpha, stroke_alpha)
    nc = _get_nc()
    in_maps = []
    for c in range(NCORES):
        sl = slice(c * NS, (c + 1) * NS)
        in_maps.append({"ibt": ibt[sl], "wx": wx[sl], "wy": wy[sl],
                        "jconst": jconst, "ones2": ones2})
    r = run_bass_kernel_spmd(nc, in_maps, core_ids=list(range(NCORES)))
    fill_out = np.concatenate([r.results[c]["fill_out"] for c in range(NCORES)], axis=0)
    stroke_out = np.concatenate([r.results[c]["stroke_out"] for c in range(NCORES)], axis=0)
    return fill_out, stroke_out
